# revision 37
# baseline (speedup 1.0000x reference)
"""Trainium2 Bass kernel for nn_AttCM_67396626809426.

Computation (per batch element b, C=256, H=W=64, HW=4096):
    h3 = relu(c3(relu(c2(relu(c1(x))))))           # 1x1 convs 256->64->128->256
    conv_out = c3x3_b2(relu(c3x3_b1(h3)))          # two 3x3 convs, pad 1
    q,k,v = 1x1 convs of h3
    S[j,n] = sum_c k[c,j] q[c,n]; A = softmax(S, axis=n)
    attn[c,m] = sum_j v[c,j] A[j,m]
    out = alpha*conv_out + beta*attn

Key restructurings (all exact in real arithmetic):
 *  A[j,m] = E[j,m]/Z[j] with E = exp(S), no max subtraction (softmax is
    shift-invariant; S magnitudes here are <<1 so exp cannot overflow), so
        attn = (v/Z) @ E = Vhat @ 1 + Vhat @ (E-1).
    The rank-1 term Vhat@1 is computed exactly from bf16 Vhat; the
    correction term stores (E-1) in fp8 (tiny values keep full relative
    precision) and contracts it against a 4096-scaled fp8 Vhat with
    DoubleRow fp8 matmuls (2x PE rate), scaling back at the PSUM drain.
 *  Per 128-row block of S the full row is produced in PSUM by fp8
    DoubleRow matmuls (q,k quantized to fp8; the S values are tiny sums
    over 256 products so this is accuracy-neutral here), exp'd on the
    scalar engine (row sums Z accumulate on the fly), and E-1 is staged
    to an HBM scratch.
 *  Trunk 1x1 convs run in float32r (full fp32 storage, ~2^-12 matmul
    accuracy, full PE rate); h3 lands relu'd as bf16 in a flat
    vertically-padded image layout ([2 zero rows | image | 2 zero rows])
    so every 3x3 tap is a single contiguous 512-wide read.  Horizontal
    wrap-around contamination at columns 0/63 is cancelled by per-layer
    correction terms (negated edge weights, stride-64 matmuls) added into
    PSUM before the activation.
 *  The 3x3 conv branch (bf16) is interleaved chunk-by-chunk into the
    attention row-block loop to keep the PE stream dense while ACT/DVE
    drain exps; alpha and conv biases fold into its PSUM drain.

Sharding: data-parallel over batch; core i handles batch element i (8 cores).
"""

import numpy as np
import ml_dtypes

import concourse.bass as bass
import concourse.tile as tile
from concourse import bacc
from concourse import mybir
from concourse.bass_utils import run_bass_kernel_spmd

F32 = mybir.dt.float32
F32R = mybir.dt.float32r
FP8 = mybir.dt.float8e4
BF16 = mybir.dt.bfloat16
AF = mybir.ActivationFunctionType
ALU = mybir.AluOpType
AX = mybir.AxisListType

P = 128
HW = 4096          # 64*64 pixels
IMG0 = 128         # flat padded image offset (2 zero rows)
NJB = 32           # number of 128-row attention blocks
NCH = 8            # 512-wide column chunks of HW

_bf = ml_dtypes.bfloat16


def _build(alpha: float, beta: float) -> bass.Bass:
    nc = bacc.Bacc("TRN2", target_bir_lowering=False, debug=False)

    def din(name, shape, dt=F32):
        return nc.dram_tensor(name, list(shape), dt, kind="ExternalInput").ap()

    xs_d = din("xs", [P, 2, HW], F32R)                 # x[b]: [c%128, c//128, pix]
    w1t_d = din("w1t", [P, 2, P], F32R)                # [i, ih, o(pad 64->128)］
    w2t_d = din("w2t", [P, P], F32R)                   # [i(pad 64->128), o]
    w3t_d = din("w3t", [P, 2, P], F32R)                # [i, oh, o]
    wqt_d = din("wqt", [P, 2, 2, P], BF16)       # [i, ih, oh, o]
    wkt_d = din("wkt", [P, 2, 2, P], BF16)
    wvt_d = din("wvt", [P, 2, 256], BF16)        # [i, ih, c]
    wb1t_d = din("wb1t", [P, 2, 9, 2, P], BF16)  # [i, ih, tap, oh, o]
    wb2t_d = din("wb2t", [P, 2, 9, 2, P], BF16)
    wb1n_d = din("wb1n", [P, 2, 2, 3, 2, P], BF16)  # [i, ih, edge, dy, oh, o] = -w
    wb2n_d = din("wb2n", [P, 2, 2, 3, 2, P], BF16)
    b1r_d = din("b1r", [P, 1])
    b2r_d = din("b2r", [P, 1])
    b3r_d = din("b3r", [P, 2])
    bqr_d = din("bqr", [P, 2])
    bkr_d = din("bkr", [P, 2])
    bvb_d = din("bvb", [P, 256])                 # bv broadcast across partitions
    bb1r_d = din("bb1r", [P, 2])
    abb2r_d = din("abb2r", [P, 2])               # alpha * bb2
    out_d = nc.dram_tensor("out", [P, 2, HW], F32, kind="ExternalOutput").ap()

    with tile.TileContext(nc) as tc:
        with (
            tc.tile_pool(name="const", bufs=1) as cp,
            tc.tile_pool(name="big", bufs=1) as big,
            tc.tile_pool(name="work", bufs=3) as wk,
            tc.tile_pool(name="zwork", bufs=4) as zw,
            tc.tile_pool(name="dram", bufs=1, space="DRAM") as dp,
        ):
            # ---- constants to SBUF
            def load(name, d, dt=None):
                t = cp.tile(list(d.shape), dt or d.dtype, name=name)
                nc.sync.dma_start(t[:], d[:])
                return t

            w1t = load("w1t_sb", w1t_d)
            w2t = load("w2t_sb", w2t_d)
            w3t = load("w3t_sb", w3t_d)
            wqt = load("wqt_sb", wqt_d)
            wkt = load("wkt_sb", wkt_d)
            b1r = load("b1r_sb", b1r_d)
            b2r = load("b2r_sb", b2r_d)
            b3r = load("b3r_sb", b3r_d)
            bqr = load("bqr_sb", bqr_d)
            bkr = load("bkr_sb", bkr_d)
            bvb = load("bvb_sb", bvb_d)
            bb1r = load("bb1r_sb", bb1r_d)
            abb2r = load("abb2r_sb", abb2r_d)

            e_dram = dp.tile([4, NJB, P, 1024], FP8, name="e_scratch")

            # ---- trunk: 1x1 convs (fp32), streamed per 512-pixel chunk;
            #      h3 lands relu'd in padded bf16 layout
            # flat pixel layout with 2 zero rows above and below the image:
            # flat index of pixel p = IMG0 + p
            h3p = big.tile([P, 2, 4352], BF16, name="h3p")
            nc.gpsimd.memset(h3p[:], 0.0)

            trunk_pool = tc.alloc_tile_pool(name="psT", bufs=8, space="PSUM")
            psT = trunk_pool
            for c8 in range(NCH):
                sl = bass.ts(c8, 512)
                xc = wk.tile([P, 2, 512], F32R, tag="xc", name="xc", bufs=4)
                nc.sync.dma_start(xc[:], xs_d[:, :, sl])
                ps = psT.tile([P, 512], F32, tag="pt", name="ps_c1")
                nc.tensor.matmul(ps[:], w1t[:, 0], xc[:, 0], start=True, stop=False)
                nc.tensor.matmul(ps[:], w1t[:, 1], xc[:, 1], start=False, stop=True)
                h1c = wk.tile([P, 512], F32R, tag="h1c", name="h1c", bufs=4)
                nc.scalar.activation(h1c[:], ps[:], AF.Relu, bias=b1r[:, 0:1])
                ps = psT.tile([P, 512], F32, tag="pt", name="ps_c2")
                nc.tensor.matmul(ps[:], w2t[:], h1c[:], start=True, stop=True)
                h2c = wk.tile([P, 512], F32R, tag="h2c", name="h2c", bufs=4)
                nc.vector.tensor_scalar(h2c[:], ps[:], b2r[:, 0:1], 0.0,
                                        ALU.add, ALU.max)
                for oh in range(2):
                    ps = psT.tile([P, 512], F32, tag="pt", name="ps_c3")
                    nc.tensor.matmul(ps[:], w3t[:, oh], h2c[:], start=True, stop=True)
                    nc.scalar.activation(
                        h3p[:, oh, IMG0 + c8 * 512:IMG0 + (c8 + 1) * 512], ps[:],
                        AF.Relu, bias=b3r[:, oh:oh + 1])

            # ---- q, k (bf16)
            q_sb = big.tile([P, 2, HW], FP8, name="q_sb")
            k_sb = big.tile([P, 2, HW], FP8, name="k_sb")
            for dst, wt, br in ((q_sb, wqt, bqr), (k_sb, wkt, bkr)):
                for oh in range(2):
                    for c8 in range(NCH):
                        ps = psT.tile([P, 512], F32, tag="pt", name="ps_qk")
                        sl5 = bass.ds(IMG0 + c8 * 512, 512)
                        nc.tensor.matmul(ps[:], wt[:, 0, oh], h3p[:, 0, sl5],
                                         start=True, stop=False)
                        nc.tensor.matmul(ps[:], wt[:, 1, oh], h3p[:, 1, sl5],
                                         start=False, stop=True)
                        nc.vector.tensor_scalar_add(dst[:, oh, bass.ts(c8, 512)],
                                                    ps[:], br[:, oh:oh + 1])

            trunk_pool.release()

            # big conv/v weights land while the trunk runs
            wvt = load("wvt_sb", wvt_d)
            wb1t = load("wb1t_sb", wb1t_d)
            wb2t = load("wb2t_sb", wb2t_d)
            wb1n = load("wb1n_sb", wb1n_d)
            wb2n = load("wb2n_sb", wb2n_d)

            # ---- phase A (attention row blocks) interleaved with the conv
            #      branch so the PE stream stays dense while ACT does exps
            psS = tc.alloc_tile_pool(name="psS", bufs=2, space="PSUM")
            psV = tc.alloc_tile_pool(name="psV", bufs=1, space="PSUM")
            psC = tc.alloc_tile_pool(name="psC", bufs=2, space="PSUM")
            psE = tc.alloc_tile_pool(name="psE", bufs=1, space="PSUM")
            vhatT = big.tile([P, NJB, 256], BF16, name="vhatT")
            vhatT8 = big.tile([P, NJB, 256], FP8, name="vhatT8")
            conv_s = big.tile([P, 2, HW], BF16, name="conv_s")
            ones1 = cp.tile([P, 1], BF16, name="ones1")
            nc.vector.memset(ones1[:], 1.0)
            midp = big.tile([P, 2, 4352], BF16, name="midp")
            nc.gpsimd.memset(midp[:], 0.0)

            def pscol(ps, col):
                # column `col` of an [128, 8x64] psum tile: stride-64, 8 elems
                return ps.rearrange("p (r c) -> p r c", c=64)[:, :, col]

            def colview64(ap_flat, start):
                # [start, start+64, ..., start+4032]: stride-64, 64 elements
                return ap_flat[:, start:start + 4096].rearrange(
                    "p (r c) -> p r c", c=64)[:, :, 0]

            def emit_corr(cin, wn):
                # corrections cancelling the wrapped col-0/col-63 reads:
                # corr[o, edge, y] = -sum_{ih,dy} w_edge[o,.,dy] * cin(wrap pix)
                corr = zw.tile([P, 2, 2, 64], F32, tag="corr", name="corr",
                               bufs=2)
                for oh in range(2):
                    for edge in range(2):
                        pse = psE.tile([P, 64], F32, tag="pe", name="ps_e")
                        for idx, (ih, dy) in enumerate(
                                (i, d) for i in range(2) for d in range(3)):
                            if edge == 0:
                                # col 0, kx=0 reads pixel (y+dy-1)*64 - 1
                                rhs = colview64(cin[:, ih],
                                                IMG0 + (dy - 1) * 64 - 1)
                            else:
                                # col 63, kx=2 reads pixel (y+dy)*64
                                rhs = colview64(cin[:, ih], IMG0 + dy * 64)
                            nc.tensor.matmul(pse[:], wn[:, ih, edge, dy, oh],
                                             rhs, start=(idx == 0),
                                             stop=(idx == 5))
                        nc.scalar.copy(corr[:, oh, edge], pse[:])
                return corr

            def emit_conv_chunk(lyr, oh, c8, cin, wt, corr):
                ps = psC.tile([P, 512], F32, tag="pt", name="ps_cv")
                first = True
                # main taps: contiguous 512-wide shifted reads; cols 0/63
                # pick up wrapped pixels from adjacent rows
                for ih in range(2):
                    for tap in range(9):
                        ky, kx = tap // 3, tap % 3
                        off = IMG0 + (c8 * 8 + ky - 1) * 64 + kx - 1
                        nc.tensor.matmul(ps[:], wt[:, ih, tap, oh],
                                         cin[:, ih, bass.ds(off, 512)],
                                         start=first,
                                         stop=(ih == 1 and tap == 8))
                        first = False
                r8 = bass.ds(c8 * 8, 8)
                nc.vector.tensor_add(pscol(ps, 0), pscol(ps, 0),
                                     corr[:, oh, 0, r8])
                nc.vector.tensor_add(pscol(ps, 63), pscol(ps, 63),
                                     corr[:, oh, 1, r8])
                if lyr == 0:
                    nc.vector.tensor_scalar(
                        midp[:, oh, IMG0 + c8 * 512:IMG0 + (c8 + 1) * 512],
                        ps[:], bb1r[:, oh:oh + 1], 0.0, ALU.add, ALU.max)
                else:
                    nc.vector.tensor_scalar(
                        conv_s[:, oh, bass.ts(c8, 512)], ps[:], float(alpha),
                        abb2r[:, oh:oh + 1], ALU.mult, ALU.add)

            # conv job schedule: layer 1 packed two-per-slot into jb 0..7 so
            # layer 2 (which needs all of midp for its corrections) can start
            # early; layer 2 spread one-per-slot over jb 9..24
            sched = {}
            for c8 in range(NCH):
                sched[c8] = [(0, 0, c8), (0, 1, c8)]
            for i in range(16):
                slot = 9 + round(i * 22 / 15)       # evenly over jb 9..31
                sched.setdefault(slot, []).append((1, i % 2, i // 2))
            corr1 = emit_corr(h3p, wb1n)
            corr2 = None

            for jb in range(NJB):
                e_raw = wk.tile([P, HW], BF16, tag="eraw", name="e_raw", bufs=2)
                e_sb = wk.tile([P, HW], FP8, tag="e", name="e_sb", bufs=2)
                zp = zw.tile([P, 4], F32, tag="zp", name="zp")
                for c4 in range(4):
                    ps = psS.tile([P, 1024], F32, tag="s", name="ps_s")
                    for h in range(2):
                        nc.tensor.matmul(ps[:, bass.ts(h, 512)],
                                         k_sb[:, :, bass.ts(jb, P)],
                                         q_sb[:, :, bass.ds(c4 * 1024 + h * 512, 512)],
                                         start=True, stop=True,
                                         perf_mode=mybir.MatmulPerfMode.DoubleRow)
                    nc.scalar.activation(e_raw[:, bass.ts(c4, 1024)], ps[:], AF.Exp,
                                         accum_out=zp[:, c4:c4 + 1])
                    # store E-1 in fp8: tiny values keep full relative
                    # precision there (E ~= 1); rank-1 Vhat@1 term is added
                    # back exactly in phase B
                    if c4 < 0:
                        nc.scalar.activation(e_sb[:, bass.ts(c4, 1024)],
                                             e_raw[:, bass.ts(c4, 1024)],
                                             AF.Copy, bias=-1.0)
                    else:
                        nc.vector.tensor_scalar_add(e_sb[:, bass.ts(c4, 1024)],
                                                    e_raw[:, bass.ts(c4, 1024)],
                                                    -1.0)
                    nc.sync.dma_start(e_dram[c4, jb], e_sb[:, bass.ts(c4, 1024)])
                # vT block: vT[j, c] = sum_i h3[i, j] wvT[i, c]   (+bv later)
                vt = psV.tile([P, 256], F32, tag="vt", name="ps_vt")
                slj = bass.ds(IMG0 + jb * P, P)
                nc.tensor.matmul(vt[:], h3p[:, 0, slj], wvt[:, 0], start=True, stop=False)
                nc.tensor.matmul(vt[:], h3p[:, 1, slj], wvt[:, 1], start=False, stop=True)
                z = zw.tile([P, 1], F32, tag="z", name="z")
                nc.vector.tensor_reduce(z[:], zp[:], axis=AX.X, op=ALU.add)
                rz = zw.tile([P, 1], F32, tag="rz", name="rz")
                nc.vector.reciprocal(rz[:], z[:])
                nc.vector.tensor_scalar_mul(rz[:], rz[:], float(beta))
                vtb = zw.tile([P, 256], F32, tag="vtb", name="vtb")
                nc.vector.tensor_add(vtb[:], vt[:], bvb[:])
                nc.vector.tensor_scalar_mul(vhatT[:, jb], vtb[:], rz[:])
                nc.vector.tensor_scalar_mul(rz[:], rz[:], 4096.0)
                nc.vector.tensor_scalar_mul(vhatT8[:, jb], vtb[:], rz[:])
                # conv chunks scheduled for this attention block
                for (lyr, oh, c8) in sched.get(jb, []):
                    if lyr == 0:
                        emit_conv_chunk(0, oh, c8, h3p, wb1t, corr1)
                    else:
                        if corr2 is None:
                            corr2 = emit_corr(midp, wb2n)
                        emit_conv_chunk(1, oh, c8, midp, wb2t, corr2)

            psE.release()
            psC.release()
            psV.release()
            psS.release()

            # ---- phase B: attn = attn0 + VhatT8^T @ (E-1)/4096, combine
            psA0 = tc.alloc_tile_pool(name="psA0", bufs=1, space="PSUM")
            attn0 = zw.tile([P, 2], F32, tag="attn0", name="attn0", bufs=1)
            for ch in range(2):
                a0 = psA0.tile([P, 1], F32, tag="a0", name="ps_a0")
                for jb in range(NJB):
                    nc.tensor.matmul(a0[:], vhatT[:, jb, bass.ts(ch, P)],
                                     ones1[:], start=(jb == 0),
                                     stop=(jb == NJB - 1))
                nc.vector.tensor_copy(attn0[:, ch:ch + 1], a0[:])
            psA0.release()
            psB = tc.alloc_tile_pool(name="psB", bufs=2, space="PSUM")
            for mc in range(4):
                accs = [psB.tile([P, 512], F32, tag=f"acc{i}", name=f"acc{i}")
                        for i in range(4)]
                for jg in range(NJB // 2):
                    e_t = wk.tile([P, 2, 1024], FP8, tag="eb", name="e_t", bufs=6)
                    nc.sync.dma_start(
                        e_t[:],
                        e_dram[mc, bass.ts(jg, 2)].rearrange("j p n -> p j n"))
                    for ch in range(2):
                        for sub in range(2):
                            nc.tensor.matmul(accs[ch * 2 + sub][:],
                                             vhatT8[:, bass.ts(jg, 2), bass.ts(ch, P)],
                                             e_t[:, :, bass.ts(sub, 512)],
                                             start=(jg == 0),
                                             stop=(jg == NJB // 2 - 1),
                                             perf_mode=mybir.MatmulPerfMode.DoubleRow)
                for ch in range(2):
                    for sub in range(2):
                        sl = bass.ds(mc * 1024 + sub * 512, 512)
                        o_t = wk.tile([P, 512], F32, tag="o", name="o_t")
                        nc.vector.tensor_scalar(o_t[:], accs[ch * 2 + sub][:],
                                                1.0 / 4096.0,
                                                attn0[:, ch:ch + 1],
                                                ALU.mult, ALU.add)
                        nc.vector.tensor_add(o_t[:], o_t[:], conv_s[:, ch, sl])
                        nc.sync.dma_start(out_d[:, ch, sl], o_t[:])
            psB.release()

    nc.compile()
    return nc


def _prep_consts(i):
    """Host-side weight layout prep. i: dict of f32 numpy arrays."""
    f32 = np.float32
    w1 = i["w1"].reshape(64, 256).astype(f32)
    w1t = np.zeros((P, 2, P), f32)
    w1t[:, :, :64] = w1.reshape(64, 2, P).transpose(2, 1, 0)
    b1r = np.zeros((P, 1), f32)
    b1r[:64, 0] = i["b1"]
    w2 = i["w2"].reshape(128, 64).astype(f32)
    w2t = np.zeros((P, P), f32)
    w2t[:64] = w2.T
    w3t = i["w3"].reshape(2, P, P).astype(f32).transpose(2, 0, 1).copy()
    wqt = i["wq"].reshape(2, P, 2, P).transpose(3, 2, 0, 1).astype(_bf)
    wkt = i["wk"].reshape(2, P, 2, P).transpose(3, 2, 0, 1).astype(_bf)
    wvt = i["wv"].reshape(256, 2, P).transpose(2, 1, 0).astype(_bf)

    def wb(w):
        a = w.reshape(2, P, 2, P, 3, 3).transpose(3, 2, 4, 5, 0, 1)
        return np.ascontiguousarray(a.reshape(P, 2, 9, 2, P)).astype(_bf)

    def wbn(w):
        # [i, ih, edge(kx=0, kx=2), dy, oh, o] = -w[oh*128+o, ih*128+i, dy, kx]
        a = w.reshape(2, P, 2, P, 3, 3).transpose(3, 2, 5, 4, 0, 1)
        a = a[:, :, (0, 2)]  # kx = 0 and 2
        return np.ascontiguousarray(-a).astype(_bf)

    alpha = float(i["alpha"])
    return {
        "w1t": w1t, "b1r": b1r, "w2t": w2t,
        "b2r": i["b2"].reshape(P, 1).astype(f32),
        "w3t": w3t, "b3r": i["b3"].reshape(2, P).T.astype(f32).copy(),
        "wqt": np.ascontiguousarray(wqt), "bqr": i["bq"].reshape(2, P).T.astype(f32).copy(),
        "wkt": np.ascontiguousarray(wkt), "bkr": i["bk"].reshape(2, P).T.astype(f32).copy(),
        "wvt": np.ascontiguousarray(wvt),
        "bvb": np.broadcast_to(i["bv"].astype(f32), (P, 256)).copy(),
        "wb1t": wb(i["wb1"]), "bb1r": i["bb1"].reshape(2, P).T.astype(f32).copy(),
        "wb2t": wb(i["wb2"]),
        "wb1n": wbn(i["wb1"]), "wb2n": wbn(i["wb2"]),
        "abb2r": (alpha * i["bb2"]).reshape(2, P).T.astype(f32).copy(),
    }


_CACHE: dict = {}


def _get_nc(alpha, beta):
    key = (round(float(alpha), 9), round(float(beta), 9))
    if key not in _CACHE:
        _CACHE[key] = _build(float(alpha), float(beta))
    return _CACHE[key]


def kernel(x, w1, b1, w2, b2, w3, b3, wb1, bb1, wb2, bb2,
           wq, bq, wk, bk, wv, bv, alpha, beta, _trace=False):
    inputs = dict(x=np.asarray(x, np.float32), w1=np.asarray(w1), b1=np.asarray(b1),
                  w2=np.asarray(w2), b2=np.asarray(b2), w3=np.asarray(w3),
                  b3=np.asarray(b3), wb1=np.asarray(wb1), bb1=np.asarray(bb1),
                  wb2=np.asarray(wb2), bb2=np.asarray(bb2), wq=np.asarray(wq),
                  bq=np.asarray(bq), wk=np.asarray(wk), bk=np.asarray(bk),
                  wv=np.asarray(wv), bv=np.asarray(bv), alpha=alpha, beta=beta)
    nc = _get_nc(inputs["alpha"], inputs["beta"])
    consts = _prep_consts(inputs)
    B = inputs["x"].shape[0]
    in_maps = []
    for b in range(B):
        m = dict(consts)
        m["xs"] = np.ascontiguousarray(
            inputs["x"][b].reshape(2, P, HW).transpose(1, 0, 2))
        in_maps.append(m)
    res = run_bass_kernel_spmd(nc, in_maps, core_ids=list(range(B)), trace=_trace)
    out = np.empty((B, 256, 64, 64), np.float32)
    for b in range(B):
        o = res.results[b]["out"]                      # [128, 2, 4096]
        out[b] = o.transpose(1, 0, 2).reshape(256, 64, 64)
    if _trace:
        return out, res
    return out
